# revision 1
# baseline (speedup 1.0000x reference)
"""Trainium2 Bass kernel: 12-head attention (B=2, N=2048, C=768) on 8 NeuronCores.

Sharding: core c -> batch b = c // 4, head-group g = c % 4 (heads 3g..3g+2).
Per core: column-sharded QKV projection, head-sharded attention, 8-core
AllToAll to re-shard from (channels, all tokens) to (all channels, my token
slice), then the output projection on the core's 512-token slice (both batch
halves are projected; the host keeps the correct one).

Device layouts are channel-major ([C, tokens]) so the exp mask bias is a
per-partition (key) ACT bias and the gathered tensor feeds the projection
directly as matmul rhs. The softmax denominator comes from an extra all-ones
column appended to V (one PV matmul yields values + row sums); division is
reciprocal_approx_fast on a gpsimd partition-broadcast of the sums row.

The query axis is processed in two parity halves (tokens {even 256-blocks},
then {odd 256-blocks}); each half ends in its own AllToAll carrying the
first/second 256 tokens of every receiver's slice, so collective #1 and the
first projection pass overlap with the second half's attention compute.
"""

import numpy as np
import ml_dtypes

B, N, C = 2, 2048, 768
H, HD = 12, 64
HPG = 3            # heads per core
GPB = 4            # cores (head-groups) per batch
NCORES = 8
SCALE = float(HD) ** -0.5
MASK_NEG = -50.0
KCH = N // 128     # 16 key chunks
DCH = C // 128     # 6 contraction chunks
NH = N // 2        # 1024 tokens per parity half

bf = ml_dtypes.bfloat16

_cache = {}


def _build():
    import concourse.mybir as mybir
    import concourse.tile as tile
    from concourse import bacc

    fp32 = mybir.dt.float32
    bfl = mybir.dt.bfloat16
    EXP = mybir.ActivationFunctionType.Exp
    MULT = mybir.AluOpType.mult

    nc = bacc.Bacc(None, num_devices=NCORES)
    xT = nc.declare_dram_parameter("xT", [C, N], bfl, isOutput=False)
    wqk = nc.declare_dram_parameter("wqk", [C, 2 * HPG * HD], bfl, isOutput=False)
    wv = nc.declare_dram_parameter("wv", [C, HPG * HD], bfl, isOutput=False)
    wp = nc.declare_dram_parameter("wp", [C, C], bfl, isOutput=False)
    bp = nc.declare_dram_parameter("bp", [128, DCH], fp32, isOutput=False)
    mb = nc.declare_dram_parameter("mb", [128, KCH], fp32, isOutput=False)
    mf = nc.declare_dram_parameter("mf", [128, KCH], fp32, isOutput=False)
    out = nc.declare_dram_parameter("out", [C, 2 * 512], fp32, isOutput=True)

    with tile.TileContext(nc) as tc:
        with (
            tc.tile_pool(name="const", bufs=1) as cpool,
            tc.tile_pool(name="work", bufs=1) as wpool,
            tc.tile_pool(name="pp", bufs=5) as ppool,
        ):
            # ---------------- input loads (order = need order) ----------------
            mb_sb = cpool.tile([128, KCH], fp32, tag="mb")
            nc.sync.dma_start(mb_sb[:], mb[:])
            mf_sb = cpool.tile([128, KCH], fp32, tag="mf")
            nc.sync.dma_start(mf_sb[:], mf[:])
            wv_sb = cpool.tile([128, DCH, HPG * HD], bfl, tag="wv")
            nc.sync.dma_start(wv_sb[:], wv.rearrange("(o p) c -> p o c", p=128))
            wqk_sb = cpool.tile([128, DCH, 2 * HPG * HD], bfl, tag="wqk")
            nc.sync.dma_start(wqk_sb[:], wqk.rearrange("(o p) c -> p o c", p=128))
            xT_sb = cpool.tile([128, DCH, N], bfl, tag="xT")
            xT_r = xT.rearrange("(o p) t -> p o t", p=128)
            for tq in range(4):
                nc.sync.dma_start(
                    xT_sb[:, :, tq * 512 : (tq + 1) * 512],
                    xT_r[:, :, tq * 512 : (tq + 1) * 512],
                )
            wp_sb = cpool.tile([128, DCH, C], bfl, tag="wp")
            nc.sync.dma_start(wp_sb[:], wp.rearrange("(o p) c -> p o c", p=128))
            bp_sb = cpool.tile([128, DCH], fp32, tag="bp")
            nc.sync.dma_start(bp_sb[:], bp[:])

            # preload the exp table set while DMAs run
            warm = cpool.tile([1, 8], fp32, tag="warm")
            nc.vector.memset(warm[:], 0.0)
            nc.scalar.activation(warm[:], warm[:], EXP)

            qT = wpool.tile([64, HPG, N], bfl, tag="qT")
            kT = wpool.tile([64, HPG, N], bfl, tag="kT")
            V3 = wpool.tile([128, KCH, HPG, HD + 1], bfl, tag="V3")
            # wqk col order: [q_h0 k_h0 q_h1 k_h1 q_h2 k_h2]
            dests = [(qT, 0), (kT, 0), (qT, 1), (kT, 1), (qT, 2), (kT, 2)]

            def qk_mtile(qkps, m, quarters):
                for tq in quarters:
                    qk_t = qkps.tile([128, 512], fp32, tag="qk")
                    for kk in range(DCH):
                        nc.tensor.matmul(
                            qk_t[:],
                            lhsT=wqk_sb[:, kk, m * 128 : (m + 1) * 128],
                            rhs=xT_sb[:, kk, tq * 512 : (tq + 1) * 512],
                            start=(kk == 0),
                            stop=(kk == DCH - 1),
                        )
                    for half in range(2):
                        dtile, j = dests[m * 2 + half]
                        nc.vector.tensor_copy(
                            dtile[:, j, tq * 512 : (tq + 1) * 512],
                            qk_t[half * 64 : (half + 1) * 64, :],
                        )

            # ---- V projection + first QK Mtile, interleaved with xT arrival ----
            aux_cm = tc.tile_pool(name="aux", bufs=2, space="PSUM")
            aux = aux_cm.__enter__()
            qkps = aux
            for tq in range(4):
                for m in range(3):
                    qk_mtile(qkps, m, [tq])
                for i in range(4 * tq, 4 * tq + 4):
                    v_t = aux.tile([128, 512], fp32, tag="qk", name="v_t")[
                        :, : HPG * HD
                    ]
                    for kk in range(DCH):
                        nc.tensor.matmul(
                            v_t[:],
                            lhsT=xT_sb[:, kk, i * 128 : (i + 1) * 128],
                            rhs=wv_sb[:, kk, :],
                            start=(kk == 0),
                            stop=(kk == DCH - 1),
                        )
                    nc.vector.tensor_scalar_mul(
                        V3[:, i, :, 0:HD],
                        v_t[:].rearrange("p (h d) -> p h d", h=HPG),
                        mf_sb[:, i : i + 1],
                    )
                    nc.vector.tensor_copy(
                        V3[:, i, :, HD],
                        mf_sb[:, i : i + 1].to_broadcast((128, HPG)),
                    )

            # ---------------- attention (parity halves) ----------------
            sps_cm = tc.tile_pool(name="sps", bufs=2, space="PSUM")
            sps = sps_cm.__enter__()
            ops_cm = tc.tile_pool(name="ops", bufs=1, space="PSUM")
            ops = ops_cm.__enter__()

            OnA = [wpool.tile([128, NH], bfl, tag=f"OnA{q}", name=f"OnA{q}") for q in range(2)]
            OnB = [wpool.tile([64, NH], bfl, tag=f"OnB{q}", name=f"OnB{q}") for q in range(2)]

            def attention_head(h, qh, extra=None):
                q_rl = qT[0:64, h, :].rearrange("p (k par c) -> p par k c", par=2, c=256)
                o_t = ops.tile([HD + 1, NH], fp32, tag="o")
                for i in range(KCH):
                    if extra is not None and i % 2 == 0 and i // 2 < len(extra):
                        extra[i // 2]()
                    s_t = sps.tile([128, NH], fp32, tag="s")
                    for n2 in range(2):
                        nc.tensor.matmul(
                            s_t[:, n2 * 512 : (n2 + 1) * 512],
                            lhsT=kT[:, h, i * 128 : (i + 1) * 128],
                            rhs=q_rl[:, qh, 2 * n2 : 2 * n2 + 2, :],
                            start=True,
                            stop=True,
                        )
                    p_t = ppool.tile([128, NH], bfl, tag="p")
                    nc.scalar.activation(
                        p_t[:], s_t[:], EXP, bias=mb_sb[:, i : i + 1], scale=SCALE
                    )
                    for n2 in range(2):
                        nc.tensor.matmul(
                            o_t[:, n2 * 512 : (n2 + 1) * 512],
                            lhsT=V3[:, i, h, :],
                            rhs=p_t[:, n2 * 512 : (n2 + 1) * 512],
                            start=(i == 0),
                            stop=(i == KCH - 1),
                        )
                sums = wpool.tile([1, NH], fp32, tag="sums")
                nc.scalar.copy(sums[:], o_t[HD : HD + 1, :])
                o_raw = wpool.tile([HD, NH], fp32, tag=f"oraw{h}")
                nc.vector.tensor_copy(o_raw[:], o_t[0:HD, :])
                rbraw = wpool.tile([HD, NH], fp32, tag="rbraw")
                nc.gpsimd.partition_broadcast(rbraw[:], sums[:])
                rb = wpool.tile([HD, NH], fp32, tag="rb")
                nc.vector.reciprocal_approx_fast(rb[:], rbraw[:])
                dst = OnA[qh][h * 64 : (h + 1) * 64, :] if h < 2 else OnB[qh][:, :]
                return nc.vector.tensor_tensor(dst, o_raw[:], rb[:], MULT)

            def bounce_and_a2a(qh, agi, ago):
                for j in range(NCORES):
                    g = j % GPB
                    nc.sync.dma_start(
                        agi[j * 192 : j * 192 + 128, :],
                        OnA[qh][:, g * 256 : (g + 1) * 256],
                    )
                    nc.sync.dma_start(
                        agi[j * 192 + 128 : (j + 1) * 192, :],
                        OnB[qh][:, g * 256 : (g + 1) * 256],
                    )
                nc.gpsimd.collective_compute(
                    "AllToAll",
                    mybir.AluOpType.bypass,
                    replica_groups=[[0, 1, 2, 3, 4, 5, 6, 7]],
                    ins=[agi[:].opt()],
                    outs=[ago[:].opt()],
                )

            at_sb = wpool.tile([128, 2 * DCH, 512], bfl, tag="at")
            out_t = out.rearrange("(o p) t -> p o t", p=128)

            at_r = at_sb[:].rearrange("p (b k) t -> p b k t", b=2)

            def proj_pass(pjps, qh, ago, after=None):
                from concourse.bass import _add_dep_helper

                dma_i = nc.sync.dma_start(
                    at_sb[:, :, qh * 256 : (qh + 1) * 256],
                    ago.rearrange("(o p) t -> p o t", p=128),
                )
                if after is not None:
                    _add_dep_helper(
                        dma_i.ins, after.ins, sync=False, reason="late proj"
                    )
                for m in range(DCH):
                    y_ps = pjps.tile([128, 512], fp32, tag="qk", name="y_ps")
                    for kk in range(DCH):
                        mm_i = nc.tensor.matmul(
                            y_ps[:],
                            lhsT=wp_sb[:, kk, m * 128 : (m + 1) * 128],
                            rhs=at_r[:, :, kk, qh * 256 : (qh + 1) * 256],
                            start=(kk == 0),
                            stop=(kk == DCH - 1),
                        )
                        if after is not None and m == 0 and kk == 0:
                            _add_dep_helper(
                                mm_i.ins, after.ins, sync=False, reason="late proj"
                            )
                    y_sb = ppool.tile([128, 512], fp32, tag="y")
                    nc.vector.tensor_scalar_add(y_sb[:], y_ps[:], bp_sb[:, m : m + 1])
                    nc.sync.dma_start(
                        out_t[:, m, :].rearrange("p (b q) -> p b q", b=2)[
                            :, :, qh * 256 : (qh + 1) * 256
                        ],
                        y_sb[:].rearrange("p (b q) -> p b q", b=2),
                    )

            ag_in = [
                nc.dram_tensor(f"ag_in{q}", [NCORES * HPG * HD, 256], bfl)
                for q in range(2)
            ]
            ag_out = [
                nc.dram_tensor(f"ag_out{q}", [NCORES * HPG * HD, 256], bfl)
                for q in range(2)
            ]

            # ---- half 0 (even 256-token blocks); QK m=1,2 interleave between heads
            attention_head(0, 0)
            attention_head(1, 0)
            attention_head(2, 0)
            bounce_and_a2a(0, ag_in[0], ag_out[0])

            # ---- half 1 (odd blocks); A2A#0 + proj pass 0 hide under compute
            attention_head(0, 1)
            attention_head(1, 1)
            last_norm = attention_head(2, 1)
            proj_pass(aux, 0, ag_out[0], after=last_norm)
            bounce_and_a2a(1, ag_in[1], ag_out[1])
            proj_pass(aux, 1, ag_out[1])

            ops_cm.__exit__(None, None, None)
            sps_cm.__exit__(None, None, None)
            aux_cm.__exit__(None, None, None)

    nc.finalize()
    return nc


def _shard_inputs(x, mask, w_qkv, w_proj, b_proj):
    in_maps = []
    for c in range(NCORES):
        b, g = c // GPB, c % GPB
        heads = [3 * g, 3 * g + 1, 3 * g + 2]
        qk_cols = [
            base + h * HD + d for h in heads for base in (0, C) for d in range(HD)
        ]
        v_cols = [2 * C + h * HD + d for h in heads for d in range(HD)]
        mrow = mask[b].astype(np.float32)
        in_maps.append(
            {
                "xT": np.ascontiguousarray(x[b].T).astype(bf),
                "wqk": np.ascontiguousarray(w_qkv[:, qk_cols]).astype(bf),
                "wv": np.ascontiguousarray(w_qkv[:, v_cols]).astype(bf),
                "wp": w_proj.astype(bf),
                "bp": np.ascontiguousarray(
                    b_proj.astype(np.float32).reshape(DCH, 128).T
                ),
                "mb": np.ascontiguousarray(
                    np.where(mrow > 0.5, 0.0, MASK_NEG)
                    .astype(np.float32)
                    .reshape(KCH, 128)
                    .T
                ),
                "mf": np.ascontiguousarray(mrow.reshape(KCH, 128).T),
            }
        )
    return in_maps


def kernel(x, mask, w_qkv, w_proj, b_proj, _trace=False):
    from concourse.bass_utils import run_bass_kernel_spmd

    x = np.asarray(x, dtype=np.float32)
    mask = np.asarray(mask)
    w_qkv = np.asarray(w_qkv, dtype=np.float32)
    w_proj = np.asarray(w_proj, dtype=np.float32)
    b_proj = np.asarray(b_proj, dtype=np.float32)
    if "nc" not in _cache:
        _cache["nc"] = _build()
    nc = _cache["nc"]
    in_maps = _shard_inputs(x, mask, w_qkv, w_proj, b_proj)
    res = run_bass_kernel_spmd(nc, in_maps, core_ids=list(range(NCORES)), trace=_trace)
    y = np.empty((B, N, C), dtype=np.float32)
    for c in range(NCORES):
        b, g = c // GPB, c % GPB
        y[b, g * 512 : (g + 1) * 512] = np.asarray(
            res.results[c]["out"][:, b * 512 : (b + 1) * 512]
        ).T
    if _trace:
        _cache["last_exec_time_ns"] = res.exec_time_ns
        _cache["last_profile"] = res.profile_json
    return y



# revision 12
# speedup vs baseline: 1.3752x; 1.3752x over previous
"""Trainium2 Bass kernel: 12-head attention (B=2, N=2048, C=768) on 8 NeuronCores.

Sharding: core c -> batch b = c // 4, head-group g = c % 4 (heads 3g..3g+2).
Per core: column-sharded QKV projection, head-sharded attention, then a
4-core same-batch AllToAll re-shards from (3 heads, all tokens) to (all
12 heads' channels, my 512-token slice), followed by the output projection
on only this core's own 512 tokens.

Key compaction: the binary key mask zeroes ~half the keys exactly
(exp(s)*0), so the host compacts the key/value token set to the unmasked
keys (padded to a multiple of 384 with zero columns biased to -50 in the
exp). K/V projection, QK, exp and PV all run only on compacted keys --
~2x less work, numerically identical (padding contributes ~e^-50).

Pipeline design (exp on the Activation engine costs ~1113ns per
[128,1024] tile; the PE's per-chunk work is ~850ns):
  - PSUM: 3 rotating s-tiles [128,1024] (6 banks) + 1 o-accumulator
    [65,1024] (2 banks). The PE runs QK 2-3 key-chunks ahead of ACT.
  - Softmax denominator comes from an all-ones column appended to V (one PV
    matmul accumulates values + row sums).
  - Normalization (PSUM drain, partition-broadcast of sums, reciprocal,
    multiply) runs on DVE+GpSimd, never on ACT/PE.
  - Queries processed in two parity halves (even/odd 256-token blocks);
    each half ends in its own 4-core AllToAll. A2A#0 overlaps half-1
    attention; A2A#1 overlaps projection pass 0.
  - All DRAM parameters are pre-arranged host-side to partition-major
    contiguous layouts so every input DMA is sequential.
"""

import numpy as np
import ml_dtypes

B, N, C = 2, 2048, 768
H, HD = 12, 64
HPG = 3            # heads per core
GPB = 4            # cores (head-groups) per batch
NCORES = 8
SCALE = float(HD) ** -0.5
MASK_NEG = -50.0
DCH = C // 128     # 6 contraction chunks
NH = N // 2        # 1024 tokens per parity half

bf = ml_dtypes.bfloat16

_cache = {}


def _build(nkv):
    import concourse.mybir as mybir
    import concourse.tile as tile
    from concourse import bacc

    kvch = nkv // 128          # compacted key chunks
    kvb = nkv // 3             # K-projection rhs block width (<=512)

    fp32 = mybir.dt.float32
    bfl = mybir.dt.bfloat16
    EXP = mybir.ActivationFunctionType.Exp
    MULT = mybir.AluOpType.mult

    nc = bacc.Bacc(None, num_devices=NCORES)
    # partition-major contiguous layouts (see _shard_inputs)
    xt = nc.declare_dram_parameter("xt", [128, 4, DCH, 512], bfl, isOutput=False)
    xkv = nc.declare_dram_parameter("xkv", [128, DCH, nkv], bfl, isOutput=False)
    # columns: [q_h0|q_h1|q_h2|k_h0|k_h1|k_h2], 64 each
    wqk = nc.declare_dram_parameter("wqk", [128, DCH, 2 * HPG * HD], bfl, isOutput=False)
    wv = nc.declare_dram_parameter("wv", [128, DCH, HPG * HD], bfl, isOutput=False)
    wp = nc.declare_dram_parameter("wp", [128, DCH, C], bfl, isOutput=False)
    bp = nc.declare_dram_parameter("bp", [128, DCH], fp32, isOutput=False)
    mb = nc.declare_dram_parameter("mb", [128, kvch], fp32, isOutput=False)
    out = nc.declare_dram_parameter("out", [128, DCH, 2, 512], fp32, isOutput=True)

    with tile.TileContext(nc) as tc:
        with (
            tc.tile_pool(name="const", bufs=1) as cpool,
            tc.tile_pool(name="work", bufs=1) as wpool,
            tc.tile_pool(name="pp", bufs=3) as ppool,
            tc.tile_pool(name="yp", bufs=2) as ypool,
            tc.tile_pool(name="sp", bufs=3, space="PSUM") as spool,
            tc.tile_pool(name="op", bufs=1, space="PSUM") as opool,
        ):
            # ---------------- input loads (order = need order) ----------------
            mb_sb = cpool.tile([128, kvch], fp32, tag="mb")
            nc.sync.dma_start(mb_sb[:], mb[:])
            wqk_sb = cpool.tile([128, DCH, 2 * HPG * HD], bfl, tag="wqk")
            nc.sync.dma_start(wqk_sb[:], wqk[:])
            wv_sb = cpool.tile([128, DCH, HPG * HD], bfl, tag="wv")
            nc.sync.dma_start(wv_sb[:], wv[:])
            xkv_sb = cpool.tile([128, DCH, nkv], bfl, tag="xkv")
            nc.sync.dma_start(xkv_sb[:], xkv[:])
            xt_sb = cpool.tile([128, 4, DCH, 512], bfl, tag="xt")
            for tq in range(4):
                nc.sync.dma_start(xt_sb[:, tq], xt[:, tq])
            wp_sb = cpool.tile([128, DCH, C], bfl, tag="wp")
            nc.sync.dma_start(wp_sb[:], wp[:])
            bp_sb = cpool.tile([128, DCH], fp32, tag="bp")
            nc.sync.dma_start(bp_sb[:], bp[:])

            # preload the exp table set while DMAs run
            warm = cpool.tile([1, 8], fp32, tag="warm")
            nc.vector.memset(warm[:], 0.0)
            nc.scalar.activation(warm[:], warm[:], EXP)

            qT = wpool.tile([64, HPG, N], bfl, tag="qT")
            kT = wpool.tile([64, HPG, nkv], bfl, tag="kT")
            V3 = wpool.tile([128, kvch, HPG, HD + 1], bfl, tag="V3")
            # ones column for the softmax denominator (V copies skip col 64)
            nc.vector.memset(V3[:, :, :, HD], 1.0)

            # ---------------- K projection (compacted keys first) -------------
            # m-tiles: [k_h0|k_h1] (128 wide) then [k_h2] (64 wide)
            for cb in range(3):
                c0 = cb * kvb
                for mt, (w0, w1, nheads) in enumerate(
                    [(192, 320, 2), (320, 384, 1)]
                ):
                    k_t = spool.tile([128, NH], fp32, tag="s", name="k_t")
                    for kk in range(DCH):
                        nc.tensor.matmul(
                            k_t[0 : (w1 - w0), 0:kvb],
                            lhsT=wqk_sb[:, kk, w0:w1],
                            rhs=xkv_sb[:, kk, c0 : c0 + kvb],
                            start=(kk == 0),
                            stop=(kk == DCH - 1),
                        )
                    for j in range(nheads):
                        nc.vector.tensor_copy(
                            kT[:, 2 * mt + j, c0 : c0 + kvb],
                            k_t[j * 64 : (j + 1) * 64, 0:kvb],
                        )
            # ---------------- V projection ----------------
            for i in range(kvch):
                v_t = spool.tile([128, NH], fp32, tag="s", name="v_t")
                for kk in range(DCH):
                    nc.tensor.matmul(
                        v_t[:, 0 : HPG * HD],
                        lhsT=xkv_sb[:, kk, i * 128 : (i + 1) * 128],
                        rhs=wv_sb[:, kk, :],
                        start=(kk == 0),
                        stop=(kk == DCH - 1),
                    )
                nc.vector.tensor_copy(
                    V3[:, i, :, 0:HD],
                    v_t[:, 0 : HPG * HD].rearrange("p (h d) -> p h d", h=HPG),
                )
            # ---------------- Q projection ----------------
            for tq in range(4):
                for mt, (w0, w1, nheads) in enumerate([(0, 128, 2), (128, 192, 1)]):
                    q_t = spool.tile([128, NH], fp32, tag="s", name="q_t")
                    for kk in range(DCH):
                        nc.tensor.matmul(
                            q_t[0 : (w1 - w0), 0:512],
                            lhsT=wqk_sb[:, kk, w0:w1],
                            rhs=xt_sb[:, tq, kk, :],
                            start=(kk == 0),
                            stop=(kk == DCH - 1),
                        )
                    for j in range(nheads):
                        nc.vector.tensor_copy(
                            qT[:, 2 * mt + j, tq * 512 : (tq + 1) * 512],
                            q_t[j * 64 : (j + 1) * 64, 0:512],
                        )

            # ---------------- attention (parity halves) ----------------
            OnA = [wpool.tile([128, NH], bfl, tag=f"OnA{q}", name=f"OnA{q}") for q in range(2)]
            OnB = [wpool.tile([64, NH], bfl, tag=f"OnB{q}", name=f"OnB{q}") for q in range(2)]

            def attention_head(h, qh):
                # queries of this parity half: 4 blocks of 256, stride 512
                q_rl = qT[0:64, h, :].rearrange(
                    "p (k par c) -> p par k c", par=2, c=256
                )
                o_t = opool.tile([HD + 1, NH], fp32, tag="o")
                for i in range(kvch):
                    s_t = spool.tile([128, NH], fp32, tag="s", name="s_t")
                    for n2 in range(2):
                        nc.tensor.matmul(
                            s_t[:, n2 * 512 : (n2 + 1) * 512],
                            lhsT=kT[:, h, i * 128 : (i + 1) * 128],
                            rhs=q_rl[:, qh, 2 * n2 : 2 * n2 + 2, :],
                            start=True,
                            stop=True,
                        )
                    p_t = ppool.tile([128, NH], bfl, tag="p")
                    nc.scalar.activation(
                        p_t[:], s_t[:], EXP, bias=mb_sb[:, i : i + 1], scale=SCALE
                    )
                    for n2 in range(2):
                        nc.tensor.matmul(
                            o_t[:, n2 * 512 : (n2 + 1) * 512],
                            lhsT=V3[:, i, h, :],
                            rhs=p_t[:, n2 * 512 : (n2 + 1) * 512],
                            start=(i == 0),
                            stop=(i == kvch - 1),
                        )
                # normalization off the PE/ACT path: drain PSUM, build 1/sums
                # (partition_broadcast reads partition 0, so stage the sums
                # row -- PSUM row 64 -- into a base-partition-0 tile first)
                o_raw = wpool.tile([HD + 1, NH], fp32, tag=f"oraw{h}")
                nc.vector.tensor_copy(o_raw[:], o_t[:])
                sums = wpool.tile([1, NH], fp32, tag="sums")
                nc.vector.tensor_copy(sums[:], o_raw[HD : HD + 1, :])
                rbraw = wpool.tile([HD, NH], fp32, tag="rbraw")
                nc.gpsimd.partition_broadcast(rbraw[:], sums[:])
                rb = wpool.tile([HD, NH], fp32, tag="rb")
                nc.vector.reciprocal_approx_fast(rb[:], rbraw[:])
                dst = OnA[qh][h * 64 : (h + 1) * 64, :] if h < 2 else OnB[qh][:, :]
                nc.vector.tensor_tensor(dst, o_raw[0:HD, :], rb[:], MULT)

            ag_in = [
                nc.dram_tensor(f"ag_in{q}", [NCORES * HPG * HD, 256], bfl)
                for q in range(2)
            ]
            ag_out = [
                nc.dram_tensor(f"ag_out{q}", [NCORES * HPG * HD, 256], bfl)
                for q in range(2)
            ]
            GROUPS = [[0, 1, 2, 3, 4, 5, 6, 7]]

            def bounce_and_a2a(qh):
                agi = ag_in[qh]
                for j in range(NCORES):
                    g = j % GPB
                    nc.sync.dma_start(
                        agi[j * 192 : j * 192 + 128, :],
                        OnA[qh][:, g * 256 : (g + 1) * 256],
                    )
                    nc.sync.dma_start(
                        agi[j * 192 + 128 : (j + 1) * 192, :],
                        OnB[qh][:, g * 256 : (g + 1) * 256],
                    )
                nc.gpsimd.collective_compute(
                    "AllToAll",
                    mybir.AluOpType.bypass,
                    replica_groups=GROUPS,
                    ins=[agi[:].opt()],
                    outs=[ag_out[qh][:].opt()],
                )

            # at: both batches' 12-head channels for my 256-token block per half
            at_sb = wpool.tile([128, 2 * DCH, 512], bfl, tag="at")
            at_r = at_sb[:].rearrange("p (b k) t -> p b k t", b=2)

            def proj_pass(qh):
                nc.sync.dma_start(
                    at_sb[:, :, qh * 256 : (qh + 1) * 256],
                    ag_out[qh].rearrange("(o p) t -> p o t", p=128),
                )
                for m in range(DCH):
                    y_ps = spool.tile([128, NH], fp32, tag="s", name="y_ps")
                    for kk in range(DCH):
                        nc.tensor.matmul(
                            y_ps[:, 0:512],
                            lhsT=wp_sb[:, kk, m * 128 : (m + 1) * 128],
                            rhs=at_r[:, :, kk, qh * 256 : (qh + 1) * 256],
                            start=(kk == 0),
                            stop=(kk == DCH - 1),
                        )
                    y_sb = ypool.tile([128, 512], fp32, tag="y")
                    nc.vector.tensor_scalar_add(
                        y_sb[:], y_ps[:, 0:512], bp_sb[:, m : m + 1]
                    )
                    nc.sync.dma_start(
                        out[:, m, :, qh * 256 : (qh + 1) * 256],
                        y_sb[:].rearrange("p (b q) -> p b q", b=2),
                    )

            # ---- half 0 (even 256-token blocks)
            for h in range(HPG):
                attention_head(h, 0)
            bounce_and_a2a(0)

            # ---- half 1 (odd blocks); A2A#0 hides under this compute
            for h in range(HPG):
                attention_head(h, 1)
            bounce_and_a2a(1)      # A2A#1 overlaps proj pass 0
            proj_pass(0)
            proj_pass(1)

    nc.finalize()
    return nc


def _shard_inputs(x, mask, w_qkv, w_proj, b_proj, nkv):
    def pmajor(a):
        # [768, X] -> [128, DCH, X] with partition-major contiguous rows
        return np.ascontiguousarray(a.reshape(DCH, 128, -1).transpose(1, 0, 2))

    in_maps = []
    for c in range(NCORES):
        b, g = c // GPB, c % GPB
        heads = [3 * g, 3 * g + 1, 3 * g + 2]
        # [q_h0|q_h1|q_h2|k_h0|k_h1|k_h2]
        qk_cols = [
            base + h * HD + d for base in (0, C) for h in heads for d in range(HD)
        ]
        v_cols = [2 * C + h * HD + d for h in heads for d in range(HD)]
        xT = np.ascontiguousarray(x[b].T).astype(bf)  # [768, 2048]
        # compacted keys: unmasked token columns, zero-padded to nkv
        idx = np.nonzero(mask[b])[0]
        nk = len(idx)
        xkv = np.zeros((C, nkv), dtype=bf)
        xkv[:, :nk] = xT[:, idx]
        mbias = np.full(nkv, MASK_NEG, dtype=np.float32)
        mbias[:nk] = 0.0
        # [128, 4, DCH, 512]: quarter-major then contraction-chunk
        xt_l = np.ascontiguousarray(
            xT.reshape(DCH, 128, 4, 512).transpose(1, 2, 0, 3)
        )
        in_maps.append(
            {
                "xt": xt_l,
                "xkv": pmajor(xkv),
                "wqk": pmajor(w_qkv[:, qk_cols].astype(bf)),
                "wv": pmajor(w_qkv[:, v_cols].astype(bf)),
                "wp": pmajor(w_proj.astype(bf)),
                "bp": np.ascontiguousarray(
                    b_proj.astype(np.float32).reshape(DCH, 128).T
                ),
                "mb": np.ascontiguousarray(
                    mbias.reshape(nkv // 128, 128).T
                ),
            }
        )
    return in_maps


def kernel(x, mask, w_qkv, w_proj, b_proj, _trace=False):
    from concourse.bass_utils import run_bass_kernel_spmd

    x = np.asarray(x, dtype=np.float32)
    mask = np.asarray(mask)
    w_qkv = np.asarray(w_qkv, dtype=np.float32)
    w_proj = np.asarray(w_proj, dtype=np.float32)
    b_proj = np.asarray(b_proj, dtype=np.float32)
    # compacted-key capacity: multiple of 384 covering the densest batch
    maxk = max(int(mask[b].sum()) for b in range(B))
    nkv = max(1152, -(-maxk // 384) * 384)
    if _cache.get("nkv") != nkv:
        _cache["nc"] = _build(nkv)
        _cache["nkv"] = nkv
    nc = _cache["nc"]
    in_maps = _shard_inputs(x, mask, w_qkv, w_proj, b_proj, nkv)
    res = run_bass_kernel_spmd(nc, in_maps, core_ids=list(range(NCORES)), trace=_trace)
    y = np.empty((B, N, C), dtype=np.float32)
    for c in range(NCORES):
        b, g = c // GPB, c % GPB
        o = np.asarray(res.results[c]["out"])[:, :, b]  # [128, DCH, 512]
        y[b, g * 512 : (g + 1) * 512] = o.transpose(1, 0, 2).reshape(C, 512).T
    if _trace:
        _cache["last_exec_time_ns"] = res.exec_time_ns
        _cache["last_profile"] = res.profile_json
    return y
